# revision 2
# baseline (speedup 1.0000x reference)
"""AttentiveConv TRN2 kernel v2: out = softmax_n((text@We)@ctx^T) @ (ctx@W2^T).

Algebraic restructure vs v1: W2 folds into the context (ctx2 = ctx@W2^T) so
the attention GEMM writes out[N,D] directly (no resT intermediate / second
chained GEMM), and the softmax normalizer 1/Z_m (per context row m; softmax
is over queries n) folds into ctx2's rows — a [128,1024] row scale per m-tile
instead of rescaling the [128,2048] attention map.

Phases per replica (PE-serial, 786k PE cycles = 327.7us roofline):
  P1: ctx2[M,D]    = matmul(lhsT=ctxT[d,m],  rhs=W2T[d,:])   fp32r -> bf16
  P2: tempT[D,N]   = matmul(lhsT=We[d',d],   rhs=textT[d',n]) fp32r
  P3: scoresT[M,N] = matmul(lhsT=ctxT[d,m],  rhs=tempT[d,n])  fp32r
      per m-tile: exp(s - max_n) -> attn bf16 (ACT, accum Z); spill attn;
      ctx2[mt] *= 1/Z  (DVE row scale)
  P4: out[N,D]     = matmul(lhsT=attnT[m,nb], rhs=ctx2[m,:])  bf16

Stall-avoidance structure (found via CoreSim timeline analysis):
  - One top-level PSUM pool shared by all phases: slots recycle with
    per-slot deps; separate scoped pools would add pool-release cliffs
    (a released pool's zone re-use waits for ALL its reads, e.g. P4's
    first psum write would wait for the last softmax's exp).
  - Stream pools (ctm/attn/ach/osb/tx0) are top-level for the same reason.
  - Weight slabs are scoped pools (w2_sb dies at P1 end, we_sb at P2 end);
    each replica's prefetch re-lands on the same zone, whose release dep
    resolved a phase earlier, so the transfer hides under the previous
    replica's P3/P4.
  - textT chunk 0 has a dedicated resident buffer (tx0p): the rotating txp
    pool's zone only frees at P1's end, so chunk 0 could not prefetch.
  - DMA queue split (per-engine DGE queues are in-order; a sem-blocked
    dma_start head-of-line blocks everything behind it): SP = streamed input
    loads in compute order; Pool = weight prefetches + attn spills; ACT =
    out writes.
  - P4's first two nt iterations are software-pipelined: both iterations'
    mt<15 accumulation runs first (~13us of PE work) so the last m-tile's
    softmax tail (max+exp, ~4.5us past P3's last matmul) never stalls the PE.
  - P2 evicts its whole [P,KT,256] psum tile in one copy: per-dt evicts
    would WAR-stall each next dt chain behind the previous eviction read.

Measured (HW, 16-replica marginal): statistically indistinguishable from v1
(~330-340us, protocol noise +-50us per run); CoreSim marginal 329us/replica
vs v1's 368us. Relative error vs fp32 reference 4.0e-3.
"""

import sys

sys.path.insert(0, "/opt/trn_rl_repo")

from contextlib import ExitStack

import numpy as np

B, N, M, D = 8, 2048, 2048, 1024
P = 128
KT = D // P  # 8 contraction tiles
MT = M // P  # 16 m-tiles
NT = N // P  # 16 n-tiles
NCH_A = 256  # P2 n-chunk
NJ = 512  # P3 psum column chunk (partial-max granularity)
KEEP_LAST = 2  # attn m-tiles kept in SBUF across the P3->P4 boundary
CTP_BUFS = 2
TXP_BUFS = 2
ATP_BUFS = 3
ACP_BUFS = 2

import os

SPILL_ENG = os.environ.get("K2_SPILL_ENG", "gpsimd")  # attn spill queue
OUTW_ENG = os.environ.get("K2_OUTW_ENG", "scalar")  # out write queue
WGT_ENG = os.environ.get("K2_WGT_ENG", "gpsimd")  # weight prefetch queue
P4PIPE = os.environ.get("K2_P4PIPE", "1") == "1"  # interleave first two nt iters

_cache = {}


def r11(x: np.ndarray) -> np.ndarray:
    """Round fp32 to fp32r (TF32: 11 explicit mantissa bits, round-nearest-even)."""
    x = np.ascontiguousarray(x, dtype=np.float32)
    u = x.view(np.uint32).astype(np.uint64)
    bias = ((u >> 12) & 1) + 0x7FF
    u = (u + bias) & np.uint64(0xFFFFF000)
    return u.astype(np.uint32).view(np.float32).reshape(x.shape)


def _build(replicas=1, phases="ABCD"):
    import concourse.bass as bass  # noqa: F401
    import concourse.mybir as mybir
    import concourse.tile as tile
    from concourse import bacc

    f32 = mybir.dt.float32
    f32r = mybir.dt.float32r
    bf16 = mybir.dt.bfloat16

    nc = bacc.Bacc(None, target_bir_lowering=False)

    textT_d = nc.declare_dram_parameter("textT", [D, N], f32r, isOutput=False)
    ctxT_d = nc.declare_dram_parameter("ctxT", [D, M], f32r, isOutput=False)
    we_d = nc.declare_dram_parameter("we", [D, D], f32r, isOutput=False)
    w2T_d = nc.declare_dram_parameter("w2T", [D, D], f32r, isOutput=False)
    out_d = nc.declare_dram_parameter("out", [N, D], f32, isOutput=True)
    attn_sc = nc.dram_tensor("attn_sc", [MT, P, N], bf16)

    with tile.TileContext(nc) as tc, ExitStack() as top:
        consts = top.enter_context(tc.tile_pool(name="consts", bufs=1))
        ctx2 = consts.tile([P, MT, D], bf16)  # 32KB/p
        tempT = consts.tile([P, KT, N], f32r)  # 64KB/p
        tx0 = consts.tile([P, KT, NCH_A], f32r)  # 8KB/p: P2 chunk-0 staging

        ctp = top.enter_context(tc.tile_pool(name="ctp", bufs=CTP_BUFS))
        atp = top.enter_context(tc.tile_pool(name="atp", bufs=ATP_BUFS))
        acp = top.enter_context(tc.tile_pool(name="acp", bufs=ACP_BUFS))
        outp = top.enter_context(tc.tile_pool(name="outp", bufs=2))
        psx = top.enter_context(tc.tile_pool(name="psx", bufs=2, space="PSUM"))

        pools = (ctx2, tempT, tx0, ctp, atp, acp, outp, psx)
        for _rep in range(replicas):
            _emit_pipeline(
                nc, tc, mybir, f32, f32r, bf16, phases, pools,
                textT_d, ctxT_d, we_d, w2T_d, out_d, attn_sc,
            )

    nc.compile()
    return nc


def _emit_pipeline(
    nc, tc, mybir, f32, f32r, bf16, phases, pools,
    textT_d, ctxT_d, we_d, w2T_d, out_d, attn_sc,
):
    ctx2, tempT, tx0, ctp, atp, acp, outp, psx = pools
    X = mybir.AxisListType.X
    ctxT_ap = ctxT_d[:].rearrange("(kt p) m -> p kt m", p=P)
    textT_ap = textT_d[:].rearrange("(kt p) n -> p kt n", p=P)
    attn_ap = attn_sc[:].rearrange("mt p n -> p mt n")

    with tc.tile_pool(name="wpe", bufs=1) as wpe:
        we_sb = wpe.tile([P, KT, D], f32r)  # 32KB/p, dies at P2 end

        # ---- P1: ctx2 = ctx @ W2^T ----
        with tc.tile_pool(name="wp2", bufs=1) as wp2:
            w2_sb = wp2.tile([P, KT, D], f32r)  # 32KB/p, dies at P1 end
            wgt_eng = getattr(nc, WGT_ENG)
            wgt_eng.dma_start(
                w2_sb[:], w2T_d[:].rearrange("(kt p) d -> p kt d", p=P)
            )
            wgt_eng.dma_start(
                we_sb[:], we_d[:].rearrange("(kt p) d -> p kt d", p=P)
            )
            for mt in range(MT):
                ctm = ctp.tile([P, KT, P], f32r, tag="ctm", name="ctm")
                nc.sync.dma_start(ctm[:], ctxT_ap[:, :, mt * P : (mt + 1) * P])
                ps = psx.tile([P, D], f32, tag="ps", name="psW")
                for h in range(2):
                    for kt in range(KT):
                        nc.tensor.matmul(
                            ps[:, h * 512 : (h + 1) * 512],
                            ctm[:, kt],
                            w2_sb[:, kt, h * 512 : (h + 1) * 512],
                            start=(kt == 0),
                            stop=(kt == KT - 1),
                        )
                nc.vector.tensor_copy(ctx2[:, mt], ps[:])
                if mt == 0:
                    # chunk-0 text prefetch rides behind P1's first ctm load
                    nc.sync.dma_start(tx0[:], textT_ap[:, :, 0:NCH_A])

        # ---- P2: tempT = We^T-layout matmul over textT ----
        with ExitStack() as p2:
            txp = p2.enter_context(tc.tile_pool(name="txp", bufs=TXP_BUFS))
            for ch in range(N // NCH_A):
                if ch == 0:
                    tx = tx0
                else:
                    tx = txp.tile([P, KT, NCH_A], f32r, tag="tx", name="tx")
                    nc.sync.dma_start(
                        tx[:], textT_ap[:, :, ch * NCH_A : (ch + 1) * NCH_A]
                    )
                ps = psx.tile([P, KT, NCH_A], f32, tag="ps", name="psA")
                for dt in range(KT):
                    for kt in range(KT):
                        nc.tensor.matmul(
                            ps[:, dt],
                            we_sb[:, kt, dt * P : (dt + 1) * P],
                            tx[:, kt],
                            start=(kt == 0),
                            stop=(kt == KT - 1),
                        )
                # single whole-tile evict: per-dt evicts would WAR-stall the
                # next dt chain behind the previous eviction read
                nc.vector.tensor_copy(
                    tempT[:, :, ch * NCH_A : (ch + 1) * NCH_A], ps[:]
                )

    # ---- P3: scoresT per m-tile, softmax over n, fold 1/Z into ctx2 ----
    keep = {}
    with ExitStack() as p3:
        smp = p3.enter_context(tc.tile_pool(name="smp", bufs=4))
        for mt in range(MT):
            ctm = ctp.tile([P, KT, P], f32r, tag="ctm", name="ctm")
            nc.sync.dma_start(ctm[:], ctxT_ap[:, :, mt * P : (mt + 1) * P])
            psw = psx.tile([P, N], f32, tag="ps", name="psB")
            for j in range(N // NJ):
                for kt in range(KT):
                    nc.tensor.matmul(
                        psw[:, j * NJ : (j + 1) * NJ],
                        ctm[:, kt],
                        tempT[:, kt, j * NJ : (j + 1) * NJ],
                        start=(kt == 0),
                        stop=(kt == KT - 1),
                    )
            nmax = smp.tile([P, 1], f32)
            nc.vector.reduce_max(nmax[:], psw[:], axis=X, negate=True)
            attn = atp.tile([P, N], bf16, tag="attn", name="attn")
            z = smp.tile([P, 1], f32)
            nc.scalar.activation(
                attn[:],
                psw[:],
                mybir.ActivationFunctionType.Exp,
                bias=nmax[:],
                accum_out=z[:],
            )
            zinv = smp.tile([P, 1], f32)
            nc.vector.reciprocal(zinv[:], z[:])
            nc.vector.tensor_scalar_mul(ctx2[:, mt], ctx2[:, mt], zinv[:])
            keep[mt] = attn
            if mt < MT - KEEP_LAST:
                getattr(nc, SPILL_ENG).dma_start(attn_sc[mt], attn[:])

    # ---- P4: out = attnT-blocks @ ctx2 ----
    n_dram_mt = MT - KEEP_LAST
    start_nt = 0
    if P4PIPE:
        # Software-pipeline nt=0 and nt=1: run both iterations' mt<15
        # accumulation first (~13us of PE work) and defer the four mt=15
        # matmuls so the last m-tile's softmax tail (max+exp+scale, ~4.5us
        # past P3's last matmul) never stalls the PE.
        start_nt = 2
        pipe = []
        for nt in (0, 1):
            ach = acp.tile([P, MT, P], bf16, tag="ach", name="ach")
            if nt == 0:
                for mt in range(n_dram_mt):
                    nc.sync.dma_start(
                        ach[:, mt], attn_ap[:, mt, nt * P : (nt + 1) * P]
                    )
            else:
                nc.sync.dma_start(
                    ach[:, :n_dram_mt],
                    attn_ap[:, :n_dram_mt, nt * P : (nt + 1) * P],
                )
            ps = psx.tile([P, D], f32, tag="ps", name="psD")
            pipe.append((nt, ach, ps))
        for nt, ach, ps in pipe:
            for h in range(2):
                for mt in range(MT - 1):
                    lhsT = (
                        keep[mt][:, nt * P : (nt + 1) * P]
                        if mt >= n_dram_mt
                        else ach[:, mt]
                    )
                    nc.tensor.matmul(
                        ps[:, h * 512 : (h + 1) * 512],
                        lhsT,
                        ctx2[:, mt, h * 512 : (h + 1) * 512],
                        start=(mt == 0),
                        stop=False,
                    )
        for nt, ach, ps in pipe:
            for h in range(2):
                nc.tensor.matmul(
                    ps[:, h * 512 : (h + 1) * 512],
                    keep[MT - 1][:, nt * P : (nt + 1) * P],
                    ctx2[:, MT - 1, h * 512 : (h + 1) * 512],
                    start=False,
                    stop=True,
                )
        for nt, ach, ps in pipe:
            osb = outp.tile([P, D], f32, tag="osb", name="osb")
            nc.vector.tensor_copy(osb[:], ps[:])
            getattr(nc, OUTW_ENG).dma_start(
                out_d[:][nt * P : (nt + 1) * P, :], osb[:]
            )
    for nt in range(start_nt, NT):
        ach = acp.tile([P, MT, P], bf16, tag="ach", name="ach")
        if nt == 0:
            # per-mt loads chase P3's per-mt spills
            for mt in range(n_dram_mt):
                nc.sync.dma_start(
                    ach[:, mt], attn_ap[:, mt, nt * P : (nt + 1) * P]
                )
        else:
            nc.sync.dma_start(
                ach[:, :n_dram_mt],
                attn_ap[:, :n_dram_mt, nt * P : (nt + 1) * P],
            )
        ps = psx.tile([P, D], f32, tag="ps", name="psD")
        for h in range(2):
            for mt in range(MT):
                lhsT = (
                    keep[mt][:, nt * P : (nt + 1) * P]
                    if mt >= n_dram_mt
                    else ach[:, mt]
                )
                nc.tensor.matmul(
                    ps[:, h * 512 : (h + 1) * 512],
                    lhsT,
                    ctx2[:, mt, h * 512 : (h + 1) * 512],
                    start=(mt == 0),
                    stop=(mt == MT - 1),
                )
        osb = outp.tile([P, D], f32, tag="osb", name="osb")
        nc.vector.tensor_copy(osb[:], ps[:])
        getattr(nc, OUTW_ENG).dma_start(out_d[:][nt * P : (nt + 1) * P, :], osb[:])


def _prep_inputs(text, context, We, W2):
    """Per-core host-side shard + transpose + fp32r pre-round."""
    we_r = r11(We)
    w2T_r = r11(W2.T)
    maps = []
    for b in range(B):
        maps.append(
            {
                "textT": r11(text[b].T),
                "ctxT": r11(context[b].T),
                "we": we_r,
                "w2T": w2T_r,
            }
        )
    return maps


def kernel(text, context, We, W2, _trace=False):
    from concourse.bass_utils import run_bass_kernel_spmd

    if "nc" not in _cache:
        _cache["nc"] = _build()
    nc = _cache["nc"]
    in_maps = _prep_inputs(
        np.asarray(text), np.asarray(context), np.asarray(We), np.asarray(W2)
    )
    res = run_bass_kernel_spmd(nc, in_maps, list(range(B)), trace=_trace)
    out = np.stack([res.results[c]["out"] for c in range(B)])
    if _trace:
        return out, res
    return out
